# revision 1
# baseline (speedup 1.0000x reference)
"""Trainium2 Bass kernel for nn_EdgeEncoder (moe_routing).

Strategy
--------
Each of E edges is routed to 1 of 9 expert MLPs (4 -> 256 -> 256), then
  out = relu(concat([type_embed[tid], source_embed[sid], pv]) @ Wf + bf).

Host (numpy, cheap O(E) work):
  * scale/mask params, group edge indices by expert (base type),
  * split every expert's edges evenly over the 8 cores, padding each
    per-core expert segment to a multiple of 128 edges so all cores run
    ONE identical program (segment boundaries are compile-time constants),
  * algebraic fusions so the device does minimal work:
      - x gets a ones-row so b1 rides inside the layer-1 matmul,
      - V[t] = W2[t] @ Wf_pv (f64 host precompute) fuses layer 2 with the
        final projection: pv @ Wf_pv == h @ V[t] + const,
      - G_t = [type_embed @ Wf_t ; source_embed @ Wf_s ; b2@Wf_pv + bf]
        turns both embedding gathers and every bias into one K=20 matmul
        against the one-hot rows (ones-row coefficient carries the consts).

Device per 512-edge block (edges pre-grouped by expert, transposed):
  hT  = relu(W1e[t].T @ xT1)            2 matmuls K=5   N=512   (PSUM 2 banks)
  outT= G_t.T-rows @ uT + V[t].T-chunks @ hT    6 matmuls K=20/128 N=512 (2 banks)
  relu PSUM->SBUF, DMA outT tiles to DRAM [D, L]; host un-permutes.

Matmuls run as float32r: 1 cycle/row. A short bf16 warm-up burst raises the
PE clock gate (HAM) at kernel start; the fp32-HIGH stream itself is
discounted by the HAM and would otherwise run at 1.2 GHz throughout.
"""

import math
import os

import ml_dtypes
import numpy as np

import concourse.bacc as bacc
import concourse.bass as bass
import concourse.mybir as mybir
import concourse.tile as tile
from concourse.bass_utils import run_bass_kernel_spmd

# ---- static module configuration (mirrors the torch source) ----
T = 9            # base types ("experts")
P_MAX = 4
D = 256
N_TYPES = 14
N_SRC = 5
NCORES = 8
BLOCK = 512      # edges per device block (one PSUM bank of fp32)
GRP = 128        # edge group granularity (PE partition dim)

BASE_MAP = np.array([0, 0, 0, 1, 1, 1, 2, 2, 3, 4, 5, 6, 7, 8], dtype=np.int32)
PCOUNT = np.array([2, 2, 1, 1, 1, 1, 3, 2, 4], dtype=np.int32)
SCALES = np.ones((T, P_MAX), dtype=np.float32)
SCALES[0, :2] = [1.0, 1e-06]      # nmos  m, w
SCALES[1, :2] = [1.0, 1e-06]      # pmos  m, w
SCALES[2, 0] = 1.0                # balun rout
SCALES[3, 0] = 1000.0             # resistor r
SCALES[4, 0] = 1e-12              # capacitor c
SCALES[5, 0] = 1e-09              # inductor l
SCALES[6, :3] = [1.0, 1.0, 1.0]   # vsource dc, mag, phase
SCALES[7, :2] = [0.001, 0.001]    # isource dc, mag
SCALES[8, :4] = [1.0, 1.0, 1e9, 1.0]  # port dbm, dc, freq, num

KX = 5                            # x rows: xT(4) + ones
KU = N_TYPES + N_SRC + 1          # 20 rows: type/source one-hot + ones

_MM_DT = (mybir.dt.float32 if os.environ.get("EDGEENC_MM_DT") == "float32"
          else mybir.dt.float32r)
_F32 = mybir.dt.float32
_BF16 = mybir.dt.bfloat16
# G matmul as bf16 hi+lo pair (2x rows, full HAM credit) vs one f32r pass
_G_SPLIT = os.environ.get("EDGEENC_G_SPLIT", "0") == "1"
# dense bf16 warm-up burst: the PE HAM clock gate never un-throttles on the
# kernel's own fp32-HIGH stream, so warm it explicitly at the start
_WARM_BURST = int(os.environ.get("EDGEENC_WARM_BURST", "24"))

_PROGRAM_CACHE: dict = {}
LAST_RESULT = None  # BassKernelResults of the most recent run (for test harness)


def _layout(base_ids: np.ndarray):
    """Per-expert per-core segment sizes (multiples of GRP), identical on
    every core so one program serves all 8."""
    n_t = np.bincount(base_ids, minlength=T)
    m_t = np.zeros(T, dtype=np.int64)
    for t in range(T):
        if n_t[t] > 0:
            per_core = math.ceil(n_t[t] / NCORES)
            m_t[t] = math.ceil(per_core / GRP) * GRP
    L0 = int(m_t.sum())
    L = math.ceil(L0 / BLOCK) * BLOCK
    # fold the tail pad into the last present expert's segment
    last = int(np.nonzero(m_t)[0][-1])
    m_t[last] += L - L0
    return n_t, m_t, L


def _group_experts(m_t: np.ndarray) -> np.ndarray:
    """expert id of each 128-edge group, concatenated per expert."""
    return np.repeat(np.arange(T), (m_t // GRP))


def _build_order(base_ids: np.ndarray, n_t, m_t, L) -> np.ndarray:
    """ORD[c, j] = global edge index at per-core slot j (or -1 = pad)."""
    ORD = np.full((NCORES, L), -1, dtype=np.int64)
    off = 0
    for t in range(T):
        if m_t[t] == 0:
            continue
        seg = int(m_t[t])
        idx = np.nonzero(base_ids == t)[0]
        arr = np.full(NCORES * seg, -1, dtype=np.int64)
        arr[: idx.shape[0]] = idx
        ORD[:, off : off + seg] = arr.reshape(NCORES, seg)
        off += seg
    return ORD


def _host_inputs(type_ids, source_ids, params, ORD):
    """INX[c] = [5, L]: xT (scaled/masked) + ones row.
    INU[c] = [20, L]: type one-hot, source one-hot, ones row."""
    base_ids = BASE_MAP[type_ids]
    scales = SCALES[base_ids]                                  # [E,4]
    validp = np.arange(P_MAX)[None, :] < PCOUNT[base_ids][:, None]
    x = np.where(validp, params.astype(np.float32) / scales, 0.0).astype(np.float32)

    L = ORD.shape[1]
    INX = np.zeros((NCORES, KX, L), dtype=np.float32)
    INU = np.zeros((NCORES, KU, L), dtype=np.float32)
    valid = ORD >= 0
    ids = ORD[valid]
    tmp = np.zeros((NCORES, L, P_MAX), dtype=np.float32)
    tmp[valid] = x[ids]
    INX[:, 0:P_MAX, :] = tmp.transpose(0, 2, 1)
    INX[:, P_MAX, :] = valid
    ci, co = np.nonzero(valid)
    INU[ci, type_ids[ids], co] = 1.0
    INU[ci, N_TYPES + source_ids[ids], co] = 1.0
    INU[:, KU - 1, :] = valid
    return INX, INU


def _host_weights(type_embed, source_embed, W1, b1, W2, b2, Wf, bf):
    f = np.float32
    W1 = W1.astype(f); b1 = b1.astype(f); W2 = W2.astype(np.float64)
    b2 = b2.astype(f); Wf = Wf.astype(f); bf = bf.astype(f)
    type_embed = type_embed.astype(f); source_embed = source_embed.astype(f)

    # layer1 lhsT blocks: [5, 9*256]; block t at cols [t*256,(t+1)*256)
    W1e = np.concatenate([W1, b1[:, None, :]], axis=1)          # [9,5,256]
    W1E = np.ascontiguousarray(W1e.transpose(1, 0, 2).reshape(KX, T * D))

    Wft, Wfs, Wfp = Wf[:D], Wf[D : 2 * D], Wf[2 * D :]

    # V[t] = W2[t] @ Wf_pv (f64), fusing layer 2 with the final projection.
    # lhsT blocks: [128, 18*256]; block (t,h) = V[t][h*128:(h+1)*128,:]
    V = (W2 @ Wfp.astype(np.float64)).astype(f)                 # [9,256,256]
    VR = np.ascontiguousarray(
        V.reshape(T, 2, 128, D).transpose(2, 0, 1, 3).reshape(128, T * 2 * D)
    )

    # G_t [20,256]: type rows, source rows, const row (b2@Wf_pv + bf)
    gt = type_embed @ Wft                                       # [14,256]
    gs = source_embed @ Wfs                                     # [5,256]
    gc = b2 @ Wfp + bf[None, :]                                 # [9,256]
    G = np.stack([np.concatenate([gt, gs, gc[t : t + 1]], axis=0) for t in range(T)])
    GSB = np.ascontiguousarray(G.transpose(1, 0, 2).reshape(KU, T * D))
    # optional bf16 hi+lo split (u is one-hot, so this is ~fp32-accurate)
    GHI = GSB.astype(ml_dtypes.bfloat16)
    GLO = (GSB - GHI.astype(f)).astype(ml_dtypes.bfloat16)
    return W1E, VR, GSB, GHI, GLO


def _build_program(m_t: tuple, L: int):
    """One compiled SPMD program for the given segment layout."""
    key = (m_t, L, str(_MM_DT), _G_SPLIT, _WARM_BURST)
    if key in _PROGRAM_CACHE:
        return _PROGRAM_CACHE[key]

    group_expert = _group_experts(np.asarray(m_t, dtype=np.int64))
    NB = L // BLOCK
    GP = BLOCK // GRP  # groups per block = 4

    nc = bacc.Bacc("TRN2", target_bir_lowering=False, debug=False,
                   num_devices=NCORES)
    inx_d = nc.dram_tensor("inx", [KX, L], _MM_DT, kind="ExternalInput")
    u_dt = _BF16 if _G_SPLIT else _MM_DT
    inu_d = nc.dram_tensor("inu", [KU, L], u_dt, kind="ExternalInput")
    w1e_d = nc.dram_tensor("w1e", [KX, T * D], _MM_DT, kind="ExternalInput")
    vr_d = nc.dram_tensor("vr", [128, T * 2 * D], _MM_DT, kind="ExternalInput")
    if _G_SPLIT:
        ghi_d = nc.dram_tensor("ghi", [KU, T * D], _BF16, kind="ExternalInput")
        glo_d = nc.dram_tensor("glo", [KU, T * D], _BF16, kind="ExternalInput")
    else:
        g_d = nc.dram_tensor("gsb", [KU, T * D], _MM_DT, kind="ExternalInput")
    out_d = nc.dram_tensor("out", [D, L], _F32, kind="ExternalOutput")

    RELU = mybir.ActivationFunctionType.Relu

    with tile.TileContext(nc) as tc:
        with (
            tc.tile_pool(name="wts", bufs=1) as wts,
            tc.tile_pool(name="inp", bufs=1) as inp,
            tc.tile_pool(name="hsb", bufs=6) as hsbp,
            tc.tile_pool(name="osb", bufs=6) as osbp,
            tc.tile_pool(name="hps", bufs=4, space=bass.MemorySpace.PSUM) as hps,
            tc.tile_pool(name="ops", bufs=4, space=bass.MemorySpace.PSUM) as ops,
        ):
            w1e = wts.tile([128, T * D], _MM_DT)
            vr = wts.tile([128, T * 2 * D], _MM_DT)
            # V (2.25MB) goes on the sync queue, which is otherwise idle
            # until the first output stores ~10us in
            nc.sync.dma_start(vr[:], vr_d.ap())
            nc.vector.memset(w1e[:].bitcast(_F32), 0.0)
            nc.gpsimd.dma_start(w1e[0:KX, :], w1e_d.ap())
            if _G_SPLIT:
                ghi = wts.tile([KU, T * D], _BF16)
                glo = wts.tile([KU, T * D], _BF16)
                nc.gpsimd.dma_start(ghi[:], ghi_d.ap())
                nc.gpsimd.dma_start(glo[:], glo_d.ap())
                gmats = (ghi, glo)
            else:
                gsb = wts.tile([128, T * D], _MM_DT)
                nc.vector.memset(gsb[:].bitcast(_F32), 0.0)
                nc.gpsimd.dma_start(gsb[0:KU, :], g_d.ap())
                gmats = (gsb,)

            # bf16 scratch operands for the HAM warm-up burst
            if _WARM_BURST:
                wmw = wts.tile([128, 128], _BF16)
                wma = wts.tile([128, BLOCK], _BF16)
                nc.vector.memset(wmw[:], 0.0)
                nc.vector.memset(wma[:], 0.0)
                wmp = hps.tile([GRP, BLOCK], _F32, name="warmps", tag="hts")
                for i in range(_WARM_BURST):
                    nc.tensor.matmul(wmp[:], wmw[:], wma[:], start=True,
                                     stop=True)

            # persistent input buffers, zero-padded to K=128 partitions so
            # every matmul runs full-row (HAM activity counts whole rows)
            NIB = min(6, NB)
            xts = [inp.tile([128, BLOCK], _MM_DT, name=f"xtile{j}", tag=f"xtile{j}")
                   for j in range(NIB)]
            uts = [inp.tile([128, BLOCK], u_dt, name=f"utile{j}", tag=f"utile{j}")
                   for j in range(NIB)]
            for j in range(NIB):
                # memset rejects float32r at ISA level; bitcast to f32
                nc.vector.memset(xts[j][:].bitcast(_F32), 0.0)
                nc.vector.memset(uts[j][:].bitcast(_F32), 0.0)

            # prefetch the first blocks' inputs ahead of the 2.25MB V DMA
            for b in range(min(2, NIB)):
                nc.gpsimd.dma_start(
                    xts[b][0:KX, :], inx_d.ap()[:, b * BLOCK : (b + 1) * BLOCK])
                nc.gpsimd.dma_start(
                    uts[b][0:KU, :], inu_d.ap()[:, b * BLOCK : (b + 1) * BLOCK])

            for b in range(NB):
                g0 = b * GP
                experts = [int(group_expert[g0 + i]) for i in range(GP)]
                # runs of equal expert: (t, col0, col1) relative to block
                runs = []
                for i, t in enumerate(experts):
                    if runs and runs[-1][0] == t:
                        runs[-1] = (t, runs[-1][1], (i + 1) * GRP)
                    else:
                        runs.append((t, i * GRP, (i + 1) * GRP))

                xt_t = xts[b % NIB]
                ut_t = uts[b % NIB]
                if b >= 2:
                    nc.gpsimd.dma_start(
                        xt_t[0:KX, :], inx_d.ap()[:, b * BLOCK : (b + 1) * BLOCK])
                    nc.gpsimd.dma_start(
                        ut_t[0:KU, :], inu_d.ap()[:, b * BLOCK : (b + 1) * BLOCK])

                # ---- layer 1: hT[h] = relu(W1e[t].T @ xT1) ----
                hts = [hps.tile([GRP, BLOCK], _F32, name=f"hts{b}_{j}", tag="hts")
                       for j in range(2)]
                for (t, c0, c1) in runs:
                    for h in range(2):
                        nc.tensor.matmul(
                            hts[h][:, c0:c1],
                            w1e[:, t * D + h * GRP : t * D + (h + 1) * GRP],
                            xt_t[:, c0:c1],
                            start=True, stop=True,
                        )
                hsb = [hsbp.tile([GRP, BLOCK], _MM_DT, name=f"hsb{b}_{j}", tag="hsb")
                       for j in range(2)]
                nc.scalar.activation(hsb[0][:], hts[0][:], RELU)
                nc.scalar.activation(hsb[1][:], hts[1][:], RELU)

                # ---- fused final: outT[n,e] = G_t.T @ uT + V[t].T-chunks @ hT ----
                ots = [ops.tile([GRP, BLOCK], _F32, name=f"ots{b}_{j}", tag="ots")
                       for j in range(2)]
                for (t, c0, c1) in runs:
                    # one accumulation group per expert run per bank; MMs
                    # alternate between the two n-half banks so consecutive
                    # instructions never target the same PSUM bank
                    for g in range(2):
                        nc.tensor.matmul(
                            ots[g][:, c0:c1],
                            vr[:, (t * 2 + 0) * D + g * GRP
                               : (t * 2 + 0) * D + (g + 1) * GRP],
                            hsb[0][:, c0:c1],
                            start=True, stop=False,
                        )
                    for gmat in gmats:
                        for g in range(2):
                            nc.tensor.matmul(
                                ots[g][:, c0:c1],
                                gmat[:, t * D + g * GRP : t * D + (g + 1) * GRP],
                                ut_t[:, c0:c1],
                                start=False, stop=False,
                            )
                    for g in range(2):
                        nc.tensor.matmul(
                            ots[g][:, c0:c1],
                            vr[:, (t * 2 + 1) * D + g * GRP
                               : (t * 2 + 1) * D + (g + 1) * GRP],
                            hsb[1][:, c0:c1],
                            start=False, stop=True,
                        )
                osb = [osbp.tile([GRP, BLOCK], _F32, name=f"osb{b}_{j}", tag="osb")
                       for j in range(2)]
                nc.vector.tensor_scalar_max(osb[0][:], ots[0][:], 0.0)
                nc.vector.tensor_scalar_max(osb[1][:], ots[1][:], 0.0)
                for g in range(2):
                    nc.sync.dma_start(
                        out_d.ap()[g * GRP : (g + 1) * GRP,
                                   b * BLOCK : (b + 1) * BLOCK],
                        osb[g][:],
                    )

    nc.compile()
    _PROGRAM_CACHE[key] = nc
    return nc


def kernel(type_ids, source_ids, params, type_embed, source_embed,
           W1, b1, W2, b2, Wf, bf):
    global LAST_RESULT
    type_ids = np.asarray(type_ids, dtype=np.int32)
    source_ids = np.asarray(source_ids, dtype=np.int32)
    params = np.asarray(params, dtype=np.float32)
    E = type_ids.shape[0]

    base_ids = BASE_MAP[type_ids]
    n_t, m_t, L = _layout(base_ids)
    ORD = _build_order(base_ids, n_t, m_t, L)
    INX, INU = _host_inputs(type_ids, source_ids, params, ORD)
    W1E, VR, GSB, GHI, GLO = _host_weights(
        np.asarray(type_embed), np.asarray(source_embed),
        np.asarray(W1), np.asarray(b1), np.asarray(W2), np.asarray(b2),
        np.asarray(Wf), np.asarray(bf))

    nc = _build_program(tuple(int(v) for v in m_t), L)

    in_maps = []
    for c in range(NCORES):
        m = {"inx": np.ascontiguousarray(INX[c]), "w1e": W1E, "vr": VR}
        if _G_SPLIT:
            m["inu"] = np.ascontiguousarray(INU[c].astype(ml_dtypes.bfloat16))
            m["ghi"] = GHI
            m["glo"] = GLO
        else:
            m["inu"] = np.ascontiguousarray(INU[c])
            m["gsb"] = GSB
        in_maps.append(m)

    trace = bool(int(os.environ.get("EDGEENC_TRACE", "0")))
    res = run_bass_kernel_spmd(nc, in_maps, core_ids=list(range(NCORES)),
                               trace=trace)
    LAST_RESULT = res

    full = np.zeros((E, D), dtype=np.float32)
    for c in range(NCORES):
        sel = ORD[c] >= 0
        oc = res.results[c]["out"]                     # [D, L]
        full[ORD[c][sel]] = np.ascontiguousarray(oc[:, sel].T)
    return full

